# revision 6
# baseline (speedup 1.0000x reference)
"""Trainium2 Bass kernel for nn_CrossTransformer.

Two-stream cross-transformer: per stream s, attention uses q=qk_s,
k=qk_{1-s}, v=v_{1-s}, then FFN(Linear->LayerNorm->GELU->Linear) with
residual.

Sharding: 8 cores = 4 batches x 2 streams (pure data parallel, no
collectives). Core c handles batch c//2, stream c%2 and computes its
full [2048, 512] output slice.

Per-core dataflow (feature-major "T" = [feature partitions, tokens]):
  xsT, xoT  <- PE-transpose of bf16-cast inputs
  qT = WqkT-lhsT @ xsT   kT = WqkT-lhsT @ xoT    (feature-major)
  v  = xoT-lhsT @ WvT                            (token-major, +ones col)
  per head pair (heads 2c/2c+1 live on partitions 0:64/64:128):
    scoresT[k,q] = kT_h-lhsT @ qT_h   (row-packed across the pair)
    expT = ACT exp(0.125 * scoresT) -> bf16
    avT[65, q] = v_ext-lhsT @ expT    (row 64 = softmax denominator)
    normalize via ones-matmul partition-broadcast of 1/rowsum
  mT = WoT-lhsT @ avT_norm
  h1 = [xsT|mT]-lhsT @ Wf1T  (token-major) -> LayerNorm -> GELU
  gT = PE-transpose(g);  y = gT-lhsT @ Wf2T + bf2 + xs
"""

import sys

import numpy as np

if "/opt/trn_rl_repo" not in sys.path:
    sys.path.insert(0, "/opt/trn_rl_repo")

B, N, D = 4, 2048, 512
H, DH, FF = 8, 64, 1024
P = 128
NT = N // P   # 16 token chunks
DC = D // P   # 4 feature chunks of D
FC = FF // P  # 8 feature chunks of FF
NS = N // 512  # 4 token slices of 512

_cache: dict = {}


def _build(apply_gamma: bool, apply_beta: bool):
    import concourse.bass as bass
    import concourse.mybir as mybir
    import concourse.tile as tile
    from concourse import bacc
    from concourse.masks import make_identity

    dt = mybir.dt
    AF = mybir.ActivationFunctionType
    ALU = mybir.AluOpType
    f32 = dt.float32
    bf16 = dt.bfloat16

    nc = bacc.Bacc("TRN2", target_bir_lowering=False)

    xs_h = nc.dram_tensor("xs", [N, D], f32, kind="ExternalInput")
    xo_h = nc.dram_tensor("xo", [N, D], f32, kind="ExternalInput")
    wqk_h = nc.dram_tensor("Wqk", [D, D], f32, kind="ExternalInput")
    bqk_h = nc.dram_tensor("bqk", [D], f32, kind="ExternalInput")
    wv_h = nc.dram_tensor("Wv", [D, D], f32, kind="ExternalInput")
    bv_h = nc.dram_tensor("bv", [D], f32, kind="ExternalInput")
    wo_h = nc.dram_tensor("Wo", [D, D], f32, kind="ExternalInput")
    bo_h = nc.dram_tensor("bo", [D], f32, kind="ExternalInput")
    wf1_h = nc.dram_tensor("Wf1", [FF, FF], f32, kind="ExternalInput")
    bf1_h = nc.dram_tensor("bf1", [FF], f32, kind="ExternalInput")
    lng_h = nc.dram_tensor("ln_g", [FF], f32, kind="ExternalInput")
    lnb_h = nc.dram_tensor("ln_b", [FF], f32, kind="ExternalInput")
    wf2_h = nc.dram_tensor("Wf2", [D, FF], f32, kind="ExternalInput")
    bf2_h = nc.dram_tensor("bf2", [D], f32, kind="ExternalInput")
    y_h = nc.dram_tensor("y", [N, D], f32, kind="ExternalOutput")

    def bcast_dram(h, width):
        # [width] DRAM vector -> [128, width] partition-replicated AP
        ap = h[:]
        return bass.AP(tensor=ap.tensor, offset=ap.offset, ap=[[0, P], [1, width]])

    with tile.TileContext(nc) as tc:
        with tc.tile_pool(name="persist", bufs=1) as PS:
            # ---- persistent tiles ----
            ident = PS.tile([P, P], bf16, tag="ident")
            make_identity(nc, ident)

            xsT = PS.tile([P, DC, N], bf16, tag="xsT")
            mT = PS.tile([P, DC, N], bf16, tag="mT")

            wqkT = PS.tile([P, DC, D], bf16, tag="wqkT")
            wvT = PS.tile([P, DC, D], bf16, tag="wvT")
            woT = PS.tile([P, DC, D], bf16, tag="woT")
            wf1T = PS.tile([P, FC, FF], bf16, tag="wf1T")
            wf2T = PS.tile([P, FC, D], bf16, tag="wf2T")

            bqk_p = PS.tile([P, DC], f32, tag="bqk_p")
            bo_p = PS.tile([P, DC], f32, tag="bo_p")
            bv_bc = PS.tile([P, D], f32, tag="bv_bc")
            bf1_bc = PS.tile([P, FF], f32, tag="bf1_bc")
            bf2_bc = PS.tile([P, D], f32, tag="bf2_bc")
            ones_col = PS.tile([1, 64], f32, tag="ones_col")
            eps_t = PS.tile([P, 1], f32, tag="eps")
            nc.vector.memset(eps_t, 1e-5)

            nc.sync.dma_start(out=bqk_p, in_=bqk_h[:].rearrange("(c p) -> p c", p=P))
            nc.sync.dma_start(out=bo_p, in_=bo_h[:].rearrange("(c p) -> p c", p=P))
            nc.sync.dma_start(out=bv_bc, in_=bcast_dram(bv_h, D))
            nc.sync.dma_start(out=bf1_bc, in_=bcast_dram(bf1_h, FF))
            nc.sync.dma_start(out=bf2_bc, in_=bcast_dram(bf2_h, D))
            nc.vector.memset(ones_col, 1.0)

            if apply_gamma:
                lng_bc = PS.tile([P, FF], f32, tag="lng_bc")
                nc.sync.dma_start(out=lng_bc, in_=bcast_dram(lng_h, FF))
            if apply_beta:
                lnb_bc = PS.tile([P, FF], f32, tag="lnb_bc")
                nc.sync.dma_start(out=lnb_bc, in_=bcast_dram(lnb_h, FF))

            # ---- weight prep: load f32, cast bf16, PE-transpose ----
            with (
                tc.tile_pool(name="wstage", bufs=2) as WST,
                tc.tile_pool(name="wpsum", bufs=4, space="PSUM") as WPS,
            ):
                def prep_weight(w_h, rows, cols, dstT):
                    # w_h: [rows, cols] DRAM (row-major, out x in).
                    # dstT: [P, cols//P, rows] = w^T feature-major in bf16.
                    rc, cc = rows // P, cols // P
                    for r in range(rc):
                        st = WST.tile([P, cols], f32, tag="wst_f32")
                        nc.sync.dma_start(out=st, in_=w_h[r * P:(r + 1) * P, :])
                        stb = WST.tile([P, cols], bf16, tag="wst_bf")
                        nc.vector.tensor_copy(out=stb, in_=st)
                        for c in range(cc):
                            pt = WPS.tile([P, P], bf16, tag="wtr")
                            nc.tensor.transpose(
                                pt, stb[:, c * P:(c + 1) * P], ident
                            )
                            nc.vector.tensor_copy(
                                out=dstT[:, c, r * P:(r + 1) * P], in_=pt
                            )

                prep_weight(wqk_h, D, D, wqkT)
                prep_weight(wv_h, D, D, wvT)
                prep_weight(wo_h, D, D, woT)
                prep_weight(wf1_h, FF, FF, wf1T)
                prep_weight(wf2_h, D, FF, wf2T)

            # ---- attention-scoped tiles ----
            with tc.tile_pool(name="attn_scope", bufs=1) as AS:
                qT = AS.tile([P, DC, N], bf16, tag="qT")
                kT = AS.tile([P, DC, N], bf16, tag="kT")
                # v token-major with per-head ones column: [P, NT, 8*65]
                v_sb = AS.tile([P, NT, H * 65], bf16, tag="v_sb")
                avT = AS.tile([P, DC, N], bf16, tag="avT")

                nc.vector.memset(
                    v_sb.rearrange("p t (h w) -> p t h w", h=H)[:, :, :, 64:65], 1.0
                )

                # ---- load x, cast, transpose; project qT/kT/v ----
                with (
                    tc.tile_pool(name="xstage", bufs=3) as XST,
                    tc.tile_pool(name="xoT_pool", bufs=1) as XOP,
                    tc.psum_pool(name="ps_a", bufs=4) as PSA,
                ):
                    xoT = XOP.tile([P, DC, N], bf16, tag="xoT")

                    def load_transpose(x_h, dstT):
                        for t in range(NT):
                            st = XST.tile([P, D], f32, tag="xst_f32")
                            nc.sync.dma_start(
                                out=st, in_=x_h[t * P:(t + 1) * P, :]
                            )
                            stb = XST.tile([P, D], bf16, tag="xst_bf")
                            nc.vector.tensor_copy(out=stb, in_=st)
                            for c in range(DC):
                                pt = PSA.tile([P, P], bf16, tag="xtr")
                                nc.tensor.transpose(
                                    pt, stb[:, c * P:(c + 1) * P], ident
                                )
                                nc.vector.tensor_copy(
                                    out=dstT[:, c, t * P:(t + 1) * P], in_=pt
                                )

                    load_transpose(xs_h, xsT)
                    load_transpose(xo_h, xoT)

                    # qT / kT: [do, n] += WqkT[di, do].T @ x*T[di, n]
                    for srcT, dstT in ((xsT, qT), (xoT, kT)):
                        for doc in range(DC):
                            for tsl in range(NS):
                                ps = PSA.tile([P, 512], f32, tag="mm512")
                                for dic in range(DC):
                                    nc.tensor.matmul(
                                        ps,
                                        lhsT=wqkT[:, dic, doc * P:(doc + 1) * P],
                                        rhs=srcT[:, dic, tsl * 512:(tsl + 1) * 512],
                                        start=(dic == 0),
                                        stop=(dic == DC - 1),
                                    )
                                nc.vector.tensor_scalar_add(
                                    dstT[:, doc, tsl * 512:(tsl + 1) * 512],
                                    ps,
                                    bqk_p[:, doc:doc + 1],
                                )

                    # v: [n, do] += xoT[di, n].T @ WvT[di, do], +bias, strided
                    # into the 65-wide per-head slots.
                    for t in range(NT):
                        ps = PSA.tile([P, 512], f32, tag="mm512")
                        for dic in range(DC):
                            nc.tensor.matmul(
                                ps,
                                lhsT=xoT[:, dic, t * P:(t + 1) * P],
                                rhs=wvT[:, dic, :],
                                start=(dic == 0),
                                stop=(dic == DC - 1),
                            )
                        nc.vector.tensor_tensor(
                            v_sb.rearrange("p t (h w) -> p t h w", h=H)[
                                :, t, :, 0:64
                            ],
                            ps.rearrange("p (h d) -> p h d", h=H),
                            bv_bc.rearrange("p (h d) -> p h d", h=H),
                            ALU.add,
                        )

                # ---- attention core ----
                with (
                    tc.tile_pool(name="exp_pool", bufs=3) as EXP,
                    tc.tile_pool(name="rs_pool", bufs=2) as RSP,
                    tc.tile_pool(name="odd_pool", bufs=2) as ODD,
                    tc.psum_pool(name="ps_sc", bufs=2) as PSS,
                    tc.psum_pool(name="ps_av", bufs=1) as PSAV,
                ):
                    for c in range(DC):  # head pair (2c, 2c+1)
                        for qh in range(2):  # q halves of 1024
                            q0 = qh * 1024
                            av_e = PSAV.tile([65, 1024], f32, tag="av_e")
                            av_o = PSAV.tile([65, 1024], f32, tag="av_o")
                            for kc in range(NT):
                                ex = {}
                                for par in range(2):  # head parity in pair
                                    h = 2 * c + par
                                    lo, hi = par * 64, par * 64 + 64
                                    sc = PSS.tile([P, 1024], f32, tag="scores")
                                    for qs in range(2):
                                        nc.tensor.matmul(
                                            sc[:, qs * 512:(qs + 1) * 512],
                                            lhsT=kT[lo:hi, c, kc * P:(kc + 1) * P],
                                            rhs=qT[
                                                lo:hi,
                                                c,
                                                q0 + qs * 512:q0 + (qs + 1) * 512,
                                            ],
                                            start=True,
                                            stop=True,
                                        )
                                    et = EXP.tile([P, 1024], bf16, tag="expT")
                                    nc.scalar.activation(
                                        et, sc, AF.Exp, bias=0.0, scale=0.125
                                    )
                                    ex[par] = et
                                for par, av_ps in ((0, av_e), (1, av_o)):
                                    h = 2 * c + par
                                    for qs in range(2):
                                        nc.tensor.matmul(
                                            av_ps[:, qs * 512:(qs + 1) * 512],
                                            lhsT=v_sb[
                                                :, kc, h * 65:h * 65 + 65
                                            ],
                                            rhs=ex[par][
                                                :, qs * 512:(qs + 1) * 512
                                            ],
                                            start=(kc == 0),
                                            stop=(kc == NT - 1),
                                        )
                            # rowsums -> reciprocal -> partition-broadcast
                            rs_e = RSP.tile([1, 1024], f32, tag="rs_e")
                            rs_o = RSP.tile([1, 1024], f32, tag="rs_o")
                            nc.vector.tensor_copy(out=rs_e, in_=av_e[64:65, :])
                            nc.vector.tensor_copy(out=rs_o, in_=av_o[64:65, :])
                            rsr_e = RSP.tile([1, 1024], f32, tag="rsr_e")
                            rsr_o = RSP.tile([1, 1024], f32, tag="rsr_o")
                            nc.vector.reciprocal(rsr_e, rs_e)
                            nc.vector.reciprocal(rsr_o, rs_o)
                            for par, av_ps, rsr in (
                                (0, av_e, rsr_e),
                                (1, av_o, rsr_o),
                            ):
                                bc = PSS.tile([64, 1024], f32, tag="scores")
                                for qs in range(2):
                                    nc.tensor.matmul(
                                        bc[:, qs * 512:(qs + 1) * 512],
                                        lhsT=ones_col,
                                        rhs=rsr[0:1, qs * 512:(qs + 1) * 512],
                                        start=True,
                                        stop=True,
                                    )
                                # DVE can read only one operand from PSUM:
                                # bounce the broadcast tile to SBUF first.
                                bc_sb = RSP.tile([64, 1024], f32, tag="bc_sb")
                                nc.vector.tensor_copy(out=bc_sb, in_=bc)
                                if par == 0:
                                    nc.vector.tensor_tensor(
                                        avT[0:64, c, q0:q0 + 1024],
                                        av_ps[0:64, :],
                                        bc_sb,
                                        ALU.mult,
                                    )
                                else:
                                    sc_odd = ODD.tile([64, 1024], bf16, tag="odd")
                                    nc.vector.tensor_tensor(
                                        sc_odd, av_ps[0:64, :], bc_sb, ALU.mult
                                    )
                                    nc.sync.dma_start(
                                        out=avT[64:128, c, q0:q0 + 1024],
                                        in_=sc_odd,
                                    )

                # ---- Wo: mT[do, n] += WoT[di, do].T @ avT[di, n] ----
                with tc.psum_pool(name="ps_wo", bufs=4) as PSW:
                    for doc in range(DC):
                        for tsl in range(NS):
                            ps = PSW.tile([P, 512], f32, tag="mm512")
                            for dic in range(DC):
                                nc.tensor.matmul(
                                    ps,
                                    lhsT=woT[:, dic, doc * P:(doc + 1) * P],
                                    rhs=avT[:, dic, tsl * 512:(tsl + 1) * 512],
                                    start=(dic == 0),
                                    stop=(dic == DC - 1),
                                )
                            nc.vector.tensor_scalar_add(
                                mT[:, doc, tsl * 512:(tsl + 1) * 512],
                                ps,
                                bo_p[:, doc:doc + 1],
                            )

            # ---- FFN ----
            with (
                tc.tile_pool(name="gT_pool", bufs=1) as GTP,
                tc.tile_pool(name="fstage", bufs=3) as FST,
                tc.psum_pool(name="ps_f", bufs=4) as PSF,
                tc.psum_pool(name="ps_g", bufs=4) as PSG,
            ):
                gT = GTP.tile([P, FC, N], bf16, tag="gT")

                for t in range(NT):
                    h1 = FST.tile([P, FF], f32, tag="h1")
                    for ffs in range(2):
                        ps = PSF.tile([P, 512], f32, tag="mmf")
                        for c in range(FC):
                            lhsT = (
                                xsT[:, c, t * P:(t + 1) * P]
                                if c < DC
                                else mT[:, c - DC, t * P:(t + 1) * P]
                            )
                            nc.tensor.matmul(
                                ps,
                                lhsT=lhsT,
                                rhs=wf1T[:, c, ffs * 512:(ffs + 1) * 512],
                                start=(c == 0),
                                stop=(c == FC - 1),
                            )
                        nc.vector.tensor_tensor(
                            h1[:, ffs * 512:(ffs + 1) * 512],
                            ps,
                            bf1_bc[:, ffs * 512:(ffs + 1) * 512],
                            ALU.add,
                        )
                    # LayerNorm over FF (free dim)
                    stats = FST.tile([P, 2, 6], f32, tag="stats")
                    for sg in range(2):
                        nc.vector.bn_stats(
                            out=stats[:, sg, :],
                            in_=h1[:, sg * 512:(sg + 1) * 512],
                        )
                    mv = FST.tile([P, 2], f32, tag="mv")
                    nc.vector.bn_aggr(out=mv, in_=stats)
                    rstd = FST.tile([P, 1], f32, tag="rstd")
                    nc.scalar.activation(
                        rstd, mv[:, 1:2], AF.Sqrt, bias=eps_t, scale=1.0
                    )
                    nc.vector.reciprocal(rstd, rstd)
                    xh = FST.tile([P, FF], f32, tag="xh")
                    nc.vector.tensor_scalar(
                        xh, h1, mv[:, 0:1], rstd, ALU.subtract, ALU.mult
                    )
                    if apply_gamma:
                        nc.vector.tensor_tensor(xh, xh, lng_bc, ALU.mult)
                    if apply_beta:
                        nc.vector.tensor_tensor(xh, xh, lnb_bc, ALU.add)
                    g = FST.tile([P, FF], bf16, tag="g")
                    nc.scalar.activation(g, xh, AF.Gelu, bias=0.0, scale=1.0)
                    for fc in range(FC):
                        pt = PSG.tile([P, P], bf16, tag="gtr")
                        nc.tensor.transpose(pt, g[:, fc * P:(fc + 1) * P], ident)
                        nc.vector.tensor_copy(
                            out=gT[:, fc, t * P:(t + 1) * P], in_=pt
                        )

                # FFN2 + bias + residual
                for t in range(NT):
                    ps = PSF.tile([P, 512], f32, tag="mmf")
                    for fc in range(FC):
                        nc.tensor.matmul(
                            ps,
                            lhsT=gT[:, fc, t * P:(t + 1) * P],
                            rhs=wf2T[:, fc, :],
                            start=(fc == 0),
                            stop=(fc == FC - 1),
                        )
                    xres = FST.tile([P, D], f32, tag="xres")
                    nc.sync.dma_start(out=xres, in_=xs_h[t * P:(t + 1) * P, :])
                    yt = FST.tile([P, D], f32, tag="yt")
                    nc.vector.tensor_tensor(yt, ps, bf2_bc, ALU.add)
                    nc.vector.tensor_tensor(yt, yt, xres, ALU.add)
                    nc.sync.dma_start(out=y_h[t * P:(t + 1) * P, :], in_=yt)

    nc.compile()
    return nc


def _get_program(apply_gamma: bool, apply_beta: bool):
    key = (apply_gamma, apply_beta)
    if key not in _cache:
        _cache[key] = _build(apply_gamma, apply_beta)
    return _cache[key]


def _run(inputs, trace=False):
    from concourse.bass_utils import run_bass_kernel_spmd

    inp = {k: np.asarray(v, dtype=np.float32) for k, v in inputs.items()}

    apply_gamma = not np.all(inp["ln_g"] == 1.0)
    apply_beta = not np.all(inp["ln_b"] == 0.0)
    nc = _get_program(apply_gamma, apply_beta)

    weights = {
        k: inp[k]
        for k in (
            "Wqk", "bqk", "Wv", "bv", "Wo", "bo",
            "Wf1", "bf1", "ln_g", "ln_b", "Wf2", "bf2",
        )
    }
    x = (inp["x0"], inp["x1"])
    in_maps = []
    for core in range(8):
        b, s = core // 2, core % 2
        in_maps.append(
            {"xs": x[s][b], "xo": x[1 - s][b], **weights}
        )

    bkr = run_bass_kernel_spmd(nc, in_maps, list(range(8)), trace=trace)
    res = bkr.results
    y0 = np.stack([res[2 * b]["y"] for b in range(B)])
    y1 = np.stack([res[2 * b + 1]["y"] for b in range(B)])
    return (y0, y1), bkr


def kernel(**inputs):
    return _run(inputs, trace=False)[0]


if __name__ == "__main__":
    import reference

    inputs = reference.setup_inputs()
    out = kernel(**inputs)
    exp = reference.reference(**inputs)
    for i, (a, e) in enumerate(zip(out, exp)):
        e = np.asarray(e)
        err = np.abs(a - e).max() / np.abs(e).max()
        print(f"y{i}: rel err {err:.3e}")
